# revision 17
# baseline (speedup 1.0000x reference)
"""Trainium2 Bass kernel for AdaptConv-style GNN message passing.

Reference computation (per batch element b):
    h   = x @ W.T + b                       # [N, OUT]
    hn  = h / max(||h||_row, 1e-12)         # row-wise L2 normalize
    cos = hn @ hn.T                         # [N, N]
    out = relu((edge_weight * cos) @ h)     # [N, OUT]

Algebraic restructure used on-chip (r_p = 1/max(||h_p||, eps)):
    out[p,:] = r_p * sum_q E[p,q] * S[p,q] * hn[q,:]
with S = h h^T the UNNORMALIZED gram.  The gram operands need no
pre-normalization (no hnT materialization, half the PE transposes); the
row scale r_p is applied in a cheap bf16 epilogue against a
row-replicated copy of r built via DMA-xbar transpose + DRAM broadcast.

Sharding: pure data-parallel over batch B=8 across 8 NeuronCores.  Host
layout preprocessing: et = edge_weight[b].T (bf16), xt = x[b].T (bf16),
wt = W.T (bf16), bias = b[:,None] (fp32); output returns as
outT = [OUT, N] bf16 and the host transposes/casts to fp32.

Per-core dataflow (fp32 PSUM accumulation everywhere):
    hT[o,n]    = wt.T @ xt + bias          (PE, 2x1024 chunks + ScalarE bias)
    h tiles    = PE-transpose(hT) -> GpSimd copy to SBUF bf16 (16x)
    norms      = ScalarE Square-accum -> Sqrt -> max/recip (chunked)
    hn8 pairs  = h * r_inv -> fp8 [128, 2*OUT] interleaved (agg weights)
    per band q (16): 4 raw-gram matmuls [128,512] (1 hT-block LDW each 4)
        gates gt8 = et * S -> fp8, split: DVE-direct / GpSimd-direct /
        2x(ScalarE psum->bf16 copy -> DVE bf16 mul)
    per band pair: 4 DoubleRow fp8 matmuls accumulate outT [OUT,2048]
    epilogue: ScalarE relu -> DVE *rrep (bf16) -> DMA out per 512 chunk

The PE is fed dummy warm-up transposes first so its DVFS p-state ramps
to 2.4 GHz before the gram matmuls start, and is kept busy end-to-end.
"""

import ml_dtypes
import numpy as np

import concourse.bass as bass
import concourse.mybir as mybir
import concourse.tile as tile
from concourse import bacc
from concourse.bass_utils import run_bass_kernel_spmd
from concourse.masks import make_identity

B, N, IN, OUT = 8, 2048, 128, 128
NQ = N // 128          # 16 row bands
NP = NQ // 2           # 8 band pairs (DoubleRow)
FP32 = mybir.dt.float32
BF16 = mybir.dt.bfloat16
FP8 = mybir.dt.float8e4
AF = mybir.ActivationFunctionType
EPS = 1e-12
WARMUP = 16            # dummy PE transposes to ramp the p-state
USE_FP8 = True         # fp8 gt/hn + DoubleRow agg (2x PE on the agg matmul)

CORE_IDS = list(range(8))


def build_nc():
    from contextlib import ExitStack

    nc = bacc.Bacc("TRN2", target_bir_lowering=False, debug=False, num_devices=8)

    et = nc.dram_tensor("et", [N, N], BF16, kind="ExternalInput").ap()
    xt = nc.dram_tensor("xt", [IN, N], BF16, kind="ExternalInput").ap()
    wt = nc.dram_tensor("wt", [IN, OUT], BF16, kind="ExternalInput").ap()
    bias = nc.dram_tensor("bias", [OUT, 1], FP32, kind="ExternalInput").ap()
    out = nc.dram_tensor("out", [OUT, N], BF16, kind="ExternalOutput").ap()
    rr_d = nc.dram_tensor("rr_d", [1, N], BF16, kind="Internal").ap()

    with tile.TileContext(nc) as tc, ExitStack() as ctx:
        singles = ctx.enter_context(tc.tile_pool(name="singles", bufs=1))
        # et stream pool directly after singles so its range never aliases
        # transient scratch (no WAR deps on the stream).
        etp = ctx.enter_context(tc.tile_pool(name="etp", bufs=1))
        gt8p = ctx.enter_context(tc.tile_pool(name="gt8p", bufs=3))
        csbp = ctx.enter_context(tc.tile_pool(name="csbp", bufs=6))
        sqp = ctx.enter_context(tc.tile_pool(name="sqp", bufs=2))

        ident = singles.tile([128, 128], BF16, tag="ident")
        make_identity(nc, ident[:])

        xt_sb = singles.tile([IN, N], BF16, tag="xt_sb")
        wt_sb = singles.tile([IN, OUT], BF16, tag="wt_sb")
        bias_sb = singles.tile([OUT, 1], FP32, tag="bias_sb")
        hT = singles.tile([128, N], BF16, tag="hT")
        hrm_all = singles.tile([128, N], BF16, tag="hrm_all")
        hrm = [hrm_all[:, i * 128 : (i + 1) * 128] for i in range(NQ)]
        sq_all = singles.tile([128, N], BF16, tag="sq_all")
        gdt = FP8 if USE_FP8 else BF16
        hn8 = [
            singles.tile([128, 2 * OUT], gdt, tag=f"hn8_{i}", name=f"hn8_{i}")
            for i in range(NP)
        ]
        s_acc = singles.tile([128, NQ], FP32, tag="s_acc")
        s_nrm = singles.tile([128, NQ], FP32, tag="s_nrm")
        s_max = singles.tile([128, NQ], FP32, tag="s_max")
        r_inv = singles.tile([128, NQ], FP32, tag="r_inv")
        r_pad = singles.tile([128, 128], BF16, tag="r_pad")
        tpr = singles.tile([128, 128], BF16, tag="tpr")
        rrep = singles.tile([128, N], BF16, tag="rrep")
        out_r = singles.tile([OUT, N], BF16, tag="out_r")
        out_sb = singles.tile([OUT, N], BF16, tag="out_sb")

        # scalar Sqrt table warm + r_pad zero, off the critical path
        dumm = sqp.tile([1, 2], FP32, tag="dumm")
        nc.gpsimd.memset(dumm[:], 1.0)
        dumm2 = sqp.tile([1, 2], FP32, tag="dumm2")
        nc.scalar.activation(dumm2[:], dumm[:], AF.Sqrt)
        nc.gpsimd.memset(r_pad[:], 0.0)

        # params head the sync DMA queue, then the et band stream
        for c in range(4):
            nc.sync.dma_start(
                xt_sb[:, c * 512 : (c + 1) * 512], xt[:, c * 512 : (c + 1) * 512]
            )
        nc.sync.dma_start(wt_sb[:], wt)
        nc.sync.dma_start(bias_sb[:], bias)
        # et lives in 4 group tiles of 4 bands each; bands 0-3 get individual
        # triggers (fine-grained completion for the loop start), later groups
        # one trigger each — each sync trigger costs ~650ns and too many of
        # them starve the DMA queues early on
        etgs = [
            etp.tile([128, 4 * N], BF16, tag=f"etg{g}", name=f"etg{g}")
            for g in range(4)
        ]
        etbs = [etgs[q // 4][:, (q % 4) * N : (q % 4 + 1) * N] for q in range(NQ)]
        for q in range(4):
            nc.sync.dma_start(etbs[q], et[q * 128 : (q + 1) * 128, :])
        for g in range(1, 4):
            q0 = g * 4
            src = et[q0 * 128 : (q0 + 4) * 128, :].rearrange(
                "(j p) c -> p j c", p=128
            )
            nc.sync.dma_start(
                etgs[g][:].rearrange("p (j c) -> p j c", j=4), src
            )

        # ---------- prologue psum (scoped; closes before main pools) ----------
        with ExitStack() as pctx:
            warm = pctx.enter_context(tc.tile_pool(name="warm", bufs=1, space="PSUM"))
            hps_pool = pctx.enter_context(
                tc.tile_pool(name="hps", bufs=3, space="PSUM")
            )
            tpp = pctx.enter_context(tc.tile_pool(name="tpp", bufs=4, space="PSUM"))

            # PE p-state warm-up while waiting for xt: harmless transposes
            wtile = warm.tile([128, 128], BF16, tag="wtile")
            for _ in range(WARMUP):
                nc.tensor.transpose(wtile[:], ident[:], ident[:])

            # hT = wt.T @ xt + bias (four 512-wide chunks; first gram matmul
            # can start as soon as bias chunk 0 lands)
            hps = []
            for c in range(3):
                ps = hps_pool.tile([OUT, 512], FP32, tag="hps", name=f"hps{c}")
                nc.tensor.matmul(
                    ps[:], wt_sb[:], xt_sb[:, c * 512 : (c + 1) * 512],
                    start=True, stop=True,
                )
                hps.append(ps)
            # keep the PE p-state alive while the first bias chunk drains
            for _ in range(3):
                nc.tensor.transpose(wtile[:], ident[:], ident[:])
            ps3 = hps_pool.tile([OUT, 512], FP32, tag="hps", name="hps3")
            nc.tensor.matmul(
                ps3[:], wt_sb[:], xt_sb[:, 3 * 512 : 4 * 512], start=True, stop=True
            )
            hps.append(ps3)
            for c in range(4):
                nc.scalar.activation(
                    hT[:, c * 512 : (c + 1) * 512], hps[c][:], AF.Identity,
                    bias=bias_sb[:], scale=1.0,
                )

            # row-major h tiles: PE transpose -> DVE/ScalarE copy to SBUF bf16
            for i in range(NQ):
                tp = tpp.tile([128, 128], BF16, tag="tp", name=f"tp{i}")
                nc.tensor.transpose(tp[:], hT[:, i * 128 : (i + 1) * 128], ident[:])
                if i < 8:
                    nc.vector.tensor_copy(hrm[i], tp[:])
                else:
                    nc.scalar.copy(hrm[i], tp[:])

        # ---------- main psum pools ----------
        cosp = ctx.enter_context(tc.tile_pool(name="cosp", bufs=4, space="PSUM"))
        outp = ctx.enter_context(tc.tile_pool(name="outp", bufs=1, space="PSUM"))
        outTs = [
            outp.tile([OUT, 512], FP32, tag=f"outT{c}", name=f"outT{c}")
            for c in range(4)
        ]

        def emit_sqb(lo, hi):
            # batched row-norm^2 for bands [lo,hi): one bf16 mul (2x mode)
            # + one 3D-AP reduce over [128, k, 128] -> [128, k]
            sl = slice(lo * 128, hi * 128)
            nc.vector.tensor_mul(sq_all[:, sl], hrm_all[:, sl], hrm_all[:, sl])
            nc.vector.tensor_reduce(
                s_acc[:, lo:hi],
                sq_all[:, sl].rearrange("p (i c) -> p i c", c=128),
                mybir.AxisListType.X, mybir.AluOpType.add,
            )

        def emit_rfin(ck):
            nc.scalar.activation(s_nrm[:, ck], s_acc[:, ck], AF.Sqrt)
            nc.vector.tensor_scalar_max(s_max[:, ck], s_nrm[:, ck], EPS)
            nc.vector.reciprocal(r_inv[:, ck], s_max[:, ck])

        def emit_hn8(i, eng="s"):
            dst = hn8[i // 2][:, (i % 2) * OUT : (i % 2 + 1) * OUT]
            if eng == "s":
                nc.scalar.mul(dst, hrm[i], r_inv[:, i : i + 1])
            else:
                nc.vector.tensor_scalar_mul(dst, hrm[i], r_inv[:, i : i + 1])

        gt8s = [None] * NP

        def emit_band(q):
            """raw-gram matmuls + gates for band q."""
            if q % 2 == 0:
                gt8s[q // 2] = gt8p.tile([128, 2 * N], gdt, tag="gt8", name=f"gt8_{q//2}")
            gt = gt8s[q // 2]
            ko = (q % 2) * N
            cps = []
            for c in range(4):
                cp = cosp.tile([128, 512], FP32, tag="cps", name=f"cps{q}_{c}")
                nc.tensor.matmul(
                    cp[:], hT[:, q * 128 : (q + 1) * 128],
                    hT[:, c * 512 : (c + 1) * 512],
                    start=True, stop=True,
                )
                cps.append(cp)
            # gate split: DVE-direct is the cheapest single-op path (1x from
            # fp32 psum); the rest go ScalarE-copy -> GpSimd bf16 mul.  fp8
            # output drops DVE TT to 1x, so a copy+DVE-mul path buys nothing.
            def gate_v(c):
                nc.vector.tensor_mul(
                    gt[:, ko + c * 512 : ko + (c + 1) * 512],
                    cps[c][:], etbs[q][:, c * 512 : (c + 1) * 512],
                )

            def gate_sg(c):
                csb = csbp.tile([128, 512], BF16, tag="csb", name=f"csb{c}_{q}")
                nc.scalar.copy(csb[:], cps[c][:])
                nc.gpsimd.tensor_mul(
                    gt[:, ko + c * 512 : ko + (c + 1) * 512],
                    csb[:], etbs[q][:, c * 512 : (c + 1) * 512],
                )

            gate_v(0)
            gate_sg(1)
            if q % 2 == 0:
                gate_v(2)
            else:
                gate_sg(2)
            if q % 4 == 3:
                gate_sg(3)
            else:
                gate_v(3)

        def emit_agg(p):
            if USE_FP8:
                lhs = hn8[p][:].rearrange("q (k m) -> q k m", k=2)
                rhs = gt8s[p][:].rearrange("q (k n) -> q k n", k=2)
                for c in range(4):
                    nc.tensor.matmul(
                        outTs[c][:], lhs, rhs[:, :, c * 512 : (c + 1) * 512],
                        start=(p == 0), stop=(p == NP - 1),
                        perf_mode=mybir.MatmulPerfMode.DoubleRow,
                    )
            else:
                for ko in range(2):
                    for c in range(4):
                        nc.tensor.matmul(
                            outTs[c][:],
                            hn8[p][:, ko * OUT : (ko + 1) * OUT],
                            gt8s[p][:, ko * N + c * 512 : ko * N + (c + 1) * 512],
                            start=(p == 0 and ko == 0),
                            stop=(p == NP - 1 and ko == 1),
                        )

        emit_band(0)
        emit_sqb(0, 2)
        emit_rfin(slice(0, 2))
        emit_hn8(0, "v")
        emit_hn8(1, "v")
        emit_band(1)
        emit_sqb(2, 8)
        emit_rfin(slice(2, 8))
        emit_band(2)
        emit_agg(0)
        emit_hn8(2)
        emit_hn8(3)
        emit_band(3)
        emit_hn8(4)
        emit_hn8(5)
        emit_sqb(8, 16)
        emit_rfin(slice(8, 16))
        emit_band(4)
        emit_agg(1)
        emit_hn8(6)
        emit_hn8(7)
        emit_band(5)
        for i in range(8, 12):
            emit_hn8(i)
        emit_band(6)
        emit_agg(2)
        for i in range(12, 16):
            emit_hn8(i)
        emit_band(7)
        emit_band(8)
        emit_agg(3)
        # rrep: r_inv -> bf16 (padded) -> DMA-xbar transpose -> DRAM row ->
        # partition-broadcast DMA read back.  All off the critical path.
        nc.vector.tensor_copy(r_pad[:, 0:NQ], r_inv[:])
        nc.sync.dma_start_transpose(tpr[:], r_pad[:])
        nc.sync.dma_start(rr_d[0, :], tpr[0:NQ, :])
        nc.sync.dma_start(rrep[:], rr_d.broadcast_to([128, N]))
        emit_band(9)
        emit_band(10)
        emit_agg(4)
        emit_band(11)
        emit_band(12)
        emit_agg(5)
        emit_band(13)
        emit_band(14)
        emit_agg(6)
        emit_band(15)
        emit_agg(7)

        # epilogue: relu (ScalarE) as each outT chunk's accumulation ends,
        # then *rrep on DVE (bf16 2x), DMA out per 512 chunk
        for c in range(4):
            sl = slice(c * 512, (c + 1) * 512)
            nc.scalar.activation(out_r[:, sl], outTs[c][:], AF.Relu)
            nc.vector.tensor_mul(out_sb[:, sl], out_r[:, sl], rrep[:, sl])
        nc.sync.dma_start(out, out_sb[:])

    nc.compile()
    return nc


_NC_CACHE = None


def _get_nc():
    global _NC_CACHE
    if _NC_CACHE is None:
        _NC_CACHE = build_nc()
    return _NC_CACHE


def make_in_maps(x, edge_weight, W, b):
    x = np.asarray(x, dtype=np.float32)
    edge_weight = np.asarray(edge_weight, dtype=np.float32)
    W = np.asarray(W, dtype=np.float32)
    b = np.asarray(b, dtype=np.float32)
    wt = np.ascontiguousarray(W.T).astype(ml_dtypes.bfloat16)
    bias = np.ascontiguousarray(b.reshape(OUT, 1))
    in_maps = []
    for core in CORE_IDS:
        in_maps.append(
            {
                "et": np.ascontiguousarray(edge_weight[core].T).astype(
                    ml_dtypes.bfloat16
                ),
                "xt": np.ascontiguousarray(x[core].T).astype(ml_dtypes.bfloat16),
                "wt": wt,
                "bias": bias,
            }
        )
    return in_maps


def kernel(x, edge_weight, W, b):
    nc = _get_nc()
    in_maps = make_in_maps(x, edge_weight, W, b)
    res = run_bass_kernel_spmd(nc, in_maps, core_ids=CORE_IDS)
    out = np.stack(
        [
            np.ascontiguousarray(res.results[i]["out"].astype(np.float32).T)
            for i in range(len(CORE_IDS))
        ]
    )
    return out


# revision 18
# speedup vs baseline: 1.1134x; 1.1134x over previous
"""Trainium2 Bass kernel for AdaptConv-style GNN message passing.

Reference computation (per batch element b):
    h   = x @ W.T + b                       # [N, OUT]
    hn  = h / max(||h||_row, 1e-12)         # row-wise L2 normalize
    cos = hn @ hn.T                         # [N, N]
    out = relu((edge_weight * cos) @ h)     # [N, OUT]

Sharding: pure data-parallel over batch B=8 across the 8 NeuronCores
(no collectives).  Host-side layout preprocessing (part of the sharding
strategy): each core receives
    et   = edge_weight[b].T (bf16)  [N, N]  (so the gated matrix is produced
                                             directly in the [q, p] layout the
                                             aggregation matmul contracts over;
                                             bf16 halves the HBM stream and is
                                             well inside the accuracy budget)
    xt   = x[b].T               [IN, N]
    wt   = W.T                  [IN, OUT]
    bias = b.reshape(OUT, 1)
and returns outT = relu(out).T as [OUT, N]; the host transposes back.

On-chip dataflow per core (matmuls bf16, fp32 PSUM accumulation):
    hT[o, n]   = wt.T @ xt + bias          (TensorE + ScalarE bias)
    h_rm tiles = PE-transpose(hT)          (row-major h, agg stationary)
    norms      = Square+accum (ScalarE/VectorE) -> Sqrt/max/recip [128,16]
    hnT        = PE-transpose(h_rm * r)
    2 super-passes x 16 q-bands, two 512-col halves interleaved per band
    (et bf16 fully SBUF-resident), so each band carries two independent
    cos->gate->agg chains and both outT banks accumulate concurrently:
        cosT[q', p'] = hnT[:, q]^T @ hnT[:, half]    (PE -> PSUM)
        gT           = et[q, half] * cosT            (DVE / ACT+GpSimd /
                                                      ACT+DVE-bf16 deferred)
        outT_h      += h_rm[q]^T @ gT                (PE, PSUM accum,
                                                      trailing 6 gates)
    relu epilogue per super-pass (ScalarE) + DMA out.

PSUM: 2 outT banks + 6-deep cos pipeline = 8 banks.
"""

import ml_dtypes
import numpy as np

import concourse.bass as bass
import concourse.mybir as mybir
import concourse.tile as tile
from concourse import bacc
from concourse.bass_utils import run_bass_kernel_spmd
from concourse.masks import make_identity

B, N, IN, OUT = 8, 2048, 128, 128
NQ = N // 128
NPC = N // 512
FP32 = mybir.dt.float32
BF16 = mybir.dt.bfloat16
AF = mybir.ActivationFunctionType
EPS = 1e-12

CORE_IDS = list(range(8))


def build_nc():
    """Build + compile the single-core Bass graph (same graph runs SPMD on 8 cores)."""
    from contextlib import ExitStack

    nc = bacc.Bacc("TRN2", target_bir_lowering=False, debug=False, num_devices=8)

    et = nc.dram_tensor("et", [N, N], BF16, kind="ExternalInput").ap()
    xt = nc.dram_tensor("xt", [IN, N], BF16, kind="ExternalInput").ap()
    wt = nc.dram_tensor("wt", [IN, OUT], BF16, kind="ExternalInput").ap()
    bias = nc.dram_tensor("bias", [OUT, 1], FP32, kind="ExternalInput").ap()
    out = nc.dram_tensor("out", [OUT, N], BF16, kind="ExternalOutput").ap()

    with tile.TileContext(nc) as tc, ExitStack() as ctx:
        singles = ctx.enter_context(tc.tile_pool(name="singles", bufs=1))
        # et stream pool FIRST so its SBUF range never aliases prologue
        # scratch (WAR deps would stall the stream behind the prologue).
        etp = ctx.enter_context(tc.tile_pool(name="etp", bufs=16))
        gtp = ctx.enter_context(tc.tile_pool(name="gtp", bufs=12))
        csp = ctx.enter_context(tc.tile_pool(name="csp", bufs=6))

        ident = singles.tile([128, 128], BF16, tag="ident")
        make_identity(nc, ident[:])

        hnT = singles.tile([128, N], BF16, tag="hnT")
        hrm = [
            singles.tile([128, OUT], BF16, tag=f"hrm{i}", name=f"hrm{i}")
            for i in range(NQ)
        ]
        out_sb = singles.tile([OUT, N], BF16, tag="out_sb")
        bias_sb = singles.tile([OUT, 1], FP32, tag="bias")
        s_acc = singles.tile([128, NQ], FP32, tag="s_acc")
        s_nrm = singles.tile([128, NQ], FP32, tag="s_nrm")
        s_max = singles.tile([128, NQ], FP32, tag="s_max")
        r_inv = singles.tile([128, NQ], FP32, tag="r_inv")

        # params first on the sync ring (xt heads the prologue critical
        # path), then the et stream queues right behind
        xt_f = singles.tile([IN, N], BF16, tag="xt_f")
        nc.sync.dma_start(xt_f[:], xt)
        wt_f = singles.tile([IN, OUT], BF16, tag="wt_f")
        nc.sync.dma_start(wt_f[:], wt)
        nc.sync.dma_start(bias_sb[:], bias)

        # et stream: all 16 bands prefetched into SBUF (bf16, 64KB/partition)
        etbs = []
        for q in range(NQ):
            etb = etp.tile([128, N], BF16, tag="etb", name=f"etb{q}")
            nc.sync.dma_start(etb[:], et[q * 128 : (q + 1) * 128, :])
            etbs.append(etb)

        # ---------------- prologue: h, norms, hn (scoped pools) ----------------
        with ExitStack() as pctx:
            pro = pctx.enter_context(tc.tile_pool(name="pro", bufs=2))
            ppsum = pctx.enter_context(tc.tile_pool(name="ppsum", bufs=2, space="PSUM"))

            # warm the ScalarE Sqrt activation table off the critical path
            dummy = pro.tile([1, 2], FP32, tag="dummy")
            nc.gpsimd.memset(dummy[:], 1.0)
            dummy2 = pro.tile([1, 2], FP32, tag="dummy2")
            nc.scalar.activation(dummy2[:], dummy[:], AF.Sqrt)

            hT = pro.tile([128, N], BF16, tag="hT")
            for c in range(N // 512):
                sl = slice(c * 512, (c + 1) * 512)
                ps = ppsum.tile([OUT, 512], FP32, tag="hT_ps")
                nc.tensor.matmul(ps[:], wt_f[:], xt_f[:, sl], start=True, stop=True)
                # hT = psum + bias (per-partition bias along OUT)
                nc.scalar.activation(
                    hT[:, sl], ps[:], AF.Identity, bias=bias_sb[:], scale=1.0
                )

            # stage 1: transposes hT -> row-major h tiles; square+rowsum
            # (norm reductions split across ScalarE / VectorE)
            for i in range(NQ):
                tp = ppsum.tile([128, 128], BF16, tag="tp", bufs=4)
                nc.tensor.transpose(tp[:], hT[:, i * 128 : (i + 1) * 128], ident[:])
                nc.any.tensor_copy(hrm[i][:], tp[:])
                sq = pro.tile([128, OUT], BF16, tag="sq", bufs=4)
                if i % 2 == 0:
                    nc.scalar.activation(
                        sq[:], hrm[i][:], AF.Square, accum_out=s_acc[:, i : i + 1]
                    )
                else:
                    nc.vector.tensor_mul(sq[:], hrm[i][:], hrm[i][:])
                    nc.vector.tensor_reduce(
                        s_acc[:, i : i + 1], sq[:],
                        mybir.AxisListType.X, mybir.AluOpType.add,
                    )
            # stage 2+3 in two chunks: finalize r for tiles [0:4] as soon
            # as their squares land, so the first hnT tiles (and the main
            # loop's first cos matmuls) unblock before the norm tail
            for ck in (slice(0, 4), slice(4, NQ)):
                nc.scalar.activation(s_nrm[:, ck], s_acc[:, ck], AF.Sqrt)
                nc.vector.tensor_scalar_max(s_max[:, ck], s_nrm[:, ck], EPS)
                nc.vector.reciprocal(r_inv[:, ck], s_max[:, ck])
                for i in range(ck.start, ck.stop):
                    hn_i = pro.tile([128, OUT], BF16, tag="hn_i", bufs=4)
                    nc.vector.tensor_scalar_mul(
                        hn_i[:], hrm[i][:], r_inv[:, i : i + 1]
                    )
                    tp2 = ppsum.tile([128, 128], BF16, tag="tp", bufs=4)
                    nc.tensor.transpose(tp2[:], hn_i[:], ident[:])
                    nc.any.tensor_copy(hnT[:, i * 128 : (i + 1) * 128], tp2[:])

        # ---------------- main loop: 4 column passes x 16 bands ----------------
        cps_pool = ctx.enter_context(tc.tile_pool(name="cps", bufs=6, space="PSUM"))
        out_ps = ctx.enter_context(tc.tile_pool(name="outps", bufs=1, space="PSUM"))
        outTs = [
            out_ps.tile([OUT, 512], FP32, tag=f"outT{i}", name=f"outT{i}")
            for i in range(2)
        ]

        # Two interleaved passes per super-pass: halves A and B accumulate
        # into the two outT banks simultaneously, so every band carries two
        # independent cos->gate->agg chains (engines always have feedable
        # work) and there are only 2 pass boundaries instead of 4.
        # Per band: exactly one DVE gate + one (GpSimd or deferred
        # DVE-bf16) gate, phase-shifted so totals match the q%4 split.
        LAG = 6  # in gates (2 per band)
        for s2 in range(2):
            base = s2 * 1024
            sls = (slice(base, base + 512), slice(base + 512, base + 1024))

            def emit_agg(q, h, gt):
                nc.tensor.matmul(
                    outTs[h][:], hrm[q][:], gt[:],
                    start=(q == 0), stop=(q == NQ - 1),
                )

            pend = []
            deferred = []
            for q in range(NQ):
                cls = (q % 4, (q + 1) % 4)
                cpss = []
                for h in range(2):
                    cps = cps_pool.tile([128, 512], FP32, tag="cps")
                    nc.tensor.matmul(
                        cps[:],
                        hnT[:, q * 128 : (q + 1) * 128],
                        hnT[:, sls[h]],
                        start=True, stop=True,
                    )
                    cpss.append(cps)
                for dgt, dcsb, dh, dq in deferred:
                    nc.vector.tensor_mul(dgt[:], dcsb[:], etbs[dq][:, sls[dh]])
                deferred = []
                for h in range(2):
                    gt = gtp.tile([128, 512], BF16, tag="gt")
                    if cls[h] == 1:
                        csb = csp.tile([128, 512], BF16, tag="csb")
                        nc.scalar.copy(csb[:], cpss[h][:])
                        nc.gpsimd.tensor_mul(gt[:], csb[:], etbs[q][:, sls[h]])
                    elif cls[h] == 3:
                        csb = csp.tile([128, 512], BF16, tag="csb")
                        nc.scalar.copy(csb[:], cpss[h][:])
                        deferred.append((gt, csb, h, q))
                    else:
                        nc.vector.tensor_mul(gt[:], cpss[h][:], etbs[q][:, sls[h]])
                    pend.append((q, h, gt))
                while len(pend) > LAG:
                    emit_agg(*pend.pop(0))
            for dgt, dcsb, dh, dq in deferred:
                nc.vector.tensor_mul(dgt[:], dcsb[:], etbs[dq][:, sls[dh]])
            for item in pend:
                emit_agg(*item)

            for h in range(2):
                # per-half relu + DMA so half A ships while half B's relu runs
                nc.scalar.activation(out_sb[:, sls[h]], outTs[h][:], AF.Relu)
                nc.sync.dma_start(out[:, sls[h]], out_sb[:, sls[h]])

    nc.compile()
    return nc


_NC_CACHE = None


def _get_nc():
    global _NC_CACHE
    if _NC_CACHE is None:
        _NC_CACHE = build_nc()
    return _NC_CACHE


def make_in_maps(x, edge_weight, W, b):
    x = np.asarray(x, dtype=np.float32)
    edge_weight = np.asarray(edge_weight, dtype=np.float32)
    W = np.asarray(W, dtype=np.float32)
    b = np.asarray(b, dtype=np.float32)
    wt = np.ascontiguousarray(W.T)
    bias = np.ascontiguousarray(b.reshape(OUT, 1))
    in_maps = []
    for core in CORE_IDS:
        in_maps.append(
            {
                "et": np.ascontiguousarray(edge_weight[core].T).astype(
                    ml_dtypes.bfloat16
                ),
                "xt": np.ascontiguousarray(x[core].T).astype(ml_dtypes.bfloat16),
                "wt": wt.astype(ml_dtypes.bfloat16),
                "bias": bias,
            }
        )
    return in_maps


def kernel(x, edge_weight, W, b):
    nc = _get_nc()
    in_maps = make_in_maps(x, edge_weight, W, b)
    res = run_bass_kernel_spmd(nc, in_maps, core_ids=CORE_IDS)
    out = np.stack(
        [
            np.ascontiguousarray(res.results[i]["out"].astype(np.float32).T)
            for i in range(len(CORE_IDS))
        ]
    )
    return out

